# revision 27
# baseline (speedup 1.0000x reference)
"""DFSMN layer Trainium2 kernel (8-core SPMD, batch-parallel).

Math: per batch b,
  h = x @ W^T + b_lin                      [L, H]
  out_pre[t] = h[t] + mem[t] + fut[t]  ==  (M @ h)[t]
    with M [L, L] banded: identity + past taps (50) + future taps (5),
    taps are scalars per lag: wm = mem_w.sum(-1), wf = la_w.sum(-1).
  out = LayerNorm_H(out_pre) * gamma + beta

On device (per core = one batch):
  g' = x @ W^T + 1 (x) b_lin   (bf16 TensorE fp32 PSUM; bias folded in during
                                PSUM evacuation so M @ g' = M @ h exactly)
  pre[j] = Mdiag_j @ g'[j] + Mhalo_j @ halo_j
      halo_j = rows [128j-50 .. 128j-1] ++ [128(j+1) .. 128(j+1)+4] of g',
      assembled from neighbor tiles by two SBUF->SBUF DMAs. 2 matmuls per
      (tile, H-chunk) instead of the naive 3 band blocks + bias rank-1.
  out = (pre - mean) * rsqrt(var + eps)  (DVE bn_stats/bn_aggr)
"""
import numpy as np
import ml_dtypes

MEM, LA, EPS = 50, 5, 1e-5
B, L, D, H = 8, 2048, 1024, 2048
NCORES = 8
PT = 128              # time tile (partition dim)
TB = L // PT          # 16 time tiles
DC = D // PT          # 8 contract chunks
HN = 512              # matmul moving free dim (one PSUM bank fp32)
HC = H // HN          # 4 H chunks
HALO = MEM + LA       # 55 halo rows per tile
PAIR_HALO = False     # run halo matmuls as concurrent row-tiled pairs
HB = 64               # halo copy B partition base (row-tiled matmul pairing)
H2P = HB + HALO if PAIR_HALO else HALO  # halo tile partition count

_cached = {}
last_exec_time_ns = None


def _band_matrix(wm, wf):
    """M [L, L] fp32: out_pre = M @ h."""
    M = np.zeros((L, L), np.float32)
    idx = np.arange(L)
    M[idx, idx] = 1.0
    for t in range(L):
        if t < MEM:
            M[t, :t] += wm[:t]
        else:
            M[t, t - MEM:t] += wm
        hi = min(t + LA, L - 1)
        if hi >= t + 1:
            M[t, t + 1:hi + 1] += wf[:hi - t]
    return M


def _band_blocks(M):
    """Split M into per-tile diag blocks and halo blocks (both as lhsT).

    mdiag[k, j, t] = M[128j + t, 128j + k]          (K=128 rows of tile j)
    mhalo[k, j, t] = M[128j + t, col(j, k)] where
       col = 128j - 50 + k      for k < 50   (past halo; 0 if col < 0)
       col = 128(j+1) + k - 50  for k >= 50  (future halo; 0 if col >= L)
    """
    mdiag = np.zeros((PT, TB, PT), np.float32)
    mhalo = np.zeros((H2P, TB, PT), np.float32)
    for j in range(TB):
        r0 = j * PT
        mdiag[:, j, :] = M[r0:r0 + PT, r0:r0 + PT].T
        for k in range(MEM):
            c = r0 - MEM + k
            if c >= 0:
                mhalo[k, j, :] = M[r0:r0 + PT, c]
        for k in range(LA):
            c = (j + 1) * PT + k
            if c < L:
                mhalo[MEM + k, j, :] = M[r0:r0 + PT, c]
        # duplicate taps at partition base HB: halo matmuls for two H-chunks
        # run as a row-tiled pair (rows 0-63 / 64-127 of the PE array)
        if PAIR_HALO:
            mhalo[HB:HB + HALO, j, :] = mhalo[0:HALO, j, :]
    return mdiag, mhalo


def _dedup_ldweights(nc, mybir):
    """Remove InstLdweights that reload the stationary operand already in
    the PE array (same weights AP, only non-transpose matmuls in between).

    The tile scheduler emits one Ldweights per matmul with no reuse pass
    (walrus's ldw-opt is disabled), so back-to-back matmuls sharing a
    stationary tile pay a redundant ~53ns weight load each. Walrus emits
    non-self-loading matmuls when the explicit Ldweights is absent, so
    deleting the redundant loads is safe: only removes loads with no
    semaphore activity, and moves their scheduling deps onto the following
    matmul (which occupies the same program position).
    """
    removed = 0
    name_map = {}
    for blk in nc.m.functions[0].blocks:
        insts = list(blk.instructions)
        keep = []
        last_key = None
        pending = None  # removed LDW awaiting its following matmul
        for inst in insts:
            if isinstance(inst, mybir.InstLdweights):
                si = inst.sync_info
                clean = si is None or (not si.on_wait and not si.on_update)
                key = str(inst.ins[0]) + str(inst.tile_position)
                if (key == last_key and clean and not inst.is_transpose
                        and pending is None):
                    pending = inst
                    removed += 1
                    continue
                last_key = key
            elif isinstance(inst, mybir.InstMatmult):
                if inst.is_transpose:
                    last_key = None
                if pending is not None:
                    inst.add_sync_dependencies_from(
                        pending.sync_dependency_set_copy())
                    inst.add_nosync_dependencies_from(
                        pending.nosync_dependency_set_copy())
                    name_map[pending.name] = inst.name
                    pending = None
            elif inst.engine == mybir.EngineType.PE:
                # unknown PE-engine op: conservatively assume it can clobber
                # the loaded stationary operand
                last_key = None
                assert pending is None, "unpaired removed ldweights"
            # instructions on other engines can't touch the PE weight array
            keep.append(inst)
        assert pending is None
        if len(keep) != len(insts):
            blk.instructions[:] = keep
    if name_map:
        for blk in nc.m.functions[0].blocks:
            for inst in blk.instructions:
                inst.remap_dependency_names(name_map)
    return removed


def _build_nc(reps=1, loop_k=None):
    from concourse import bacc
    import concourse.mybir as mybir
    import concourse.tile as tile

    dt = mybir.dt.bfloat16
    f32 = mybir.dt.float32

    nc = bacc.Bacc(None, target_bir_lowering=False)
    # x shipped transposed and t-tile-major: [TB, D, PT] so tile i's lhsT
    # slices are one small contiguous region per (i, dc).
    xtT = nc.declare_dram_parameter("xtT", [TB, D, PT], dt, isOutput=False)
    wT = nc.declare_dram_parameter("wT", [D, H], dt, isOutput=False)
    mdg = nc.declare_dram_parameter("mdg", [PT, TB, PT], dt, isOutput=False)
    mhl = nc.declare_dram_parameter("mhl", [H2P, TB, PT], dt, isOutput=False)
    bvb = nc.declare_dram_parameter("bvb", [PT, H], dt, isOutput=False)
    out = nc.declare_dram_parameter("out", [L, H], f32, isOutput=True)

    with tile.TileContext(nc) as tc:
        with tc.tile_pool(name="const", bufs=1) as const, \
             tc.tile_pool(name="gpool", bufs=6) as gpool, \
             tc.tile_pool(name="hpool", bufs=5) as hpool, \
             tc.tile_pool(name="opool", bufs=2) as opool, \
             tc.tile_pool(name="ln", bufs=2) as ln, \
             tc.tile_pool(name="psg", bufs=2, space="PSUM") as psg, \
             tc.tile_pool(name="psp", bufs=1, space="PSUM") as psp:

            # Input DMAs interleaved across the two hwdge queues in
            # first-use order (wt_dc used by tile 0's dc-th matmul, xt_i by
            # tile i) so early matmuls aren't starved by later loads.
            wt_tiles = [const.tile([PT, H], dt, tag=f"wt{dc}", name=f"wt{dc}")
                        for dc in range(DC)]
            xt_tiles = [const.tile([PT, DC, PT], dt, tag=f"xt{i}",
                                   name=f"xt{i}")
                        for i in range(TB)]
            bvb_t = const.tile([PT, H], dt, tag="bvb")
            md_t = const.tile([PT, TB, PT], dt, tag="mdg")
            mh_t = const.tile([H2P, TB, PT], dt, tag="mhl")

            def load_wt(eng, dc):
                eng.dma_start(out=wt_tiles[dc],
                              in_=wT[dc * PT:(dc + 1) * PT, :])

            def load_xt(eng, i):
                eng.dma_start(
                    out=xt_tiles[i],
                    in_=xtT[i].rearrange("(dc p) t -> p dc t", p=PT))

            load_xt(nc.scalar, 0)
            for dc in range(DC):
                load_wt((nc.sync, nc.scalar)[dc % 2], dc)
            load_xt(nc.sync, 1)
            nc.scalar.dma_start(out=bvb_t, in_=bvb[:, :])
            load_xt(nc.sync, 2)
            nc.scalar.dma_start(out=md_t, in_=mdg[:, :, :])
            load_xt(nc.sync, 3)
            nc.scalar.dma_start(out=mh_t, in_=mhl[:, :, :])
            for i in range(4, TB):
                load_xt((nc.sync, nc.scalar)[i % 2], i)
            eps_t = const.tile([PT, 1], f32, tag="eps")
            nc.vector.memset(eps_t, EPS)

            if loop_k is not None:
                with tc.For_i(0, loop_k, 1):
                    _emit_body(nc, mybir, xt_tiles, wt_tiles, md_t, mh_t,
                               bvb_t, eps_t, gpool, hpool, opool, ln, psg,
                               psp, out)
            else:
                for _rep in range(reps):
                    _emit_body(nc, mybir, xt_tiles, wt_tiles, md_t, mh_t,
                               bvb_t, eps_t, gpool, hpool, opool, ln, psg,
                               psp, out)
    _dedup_ldweights(nc, mybir)
    nc.finalize()
    return nc


def _emit_body(nc, mybir, xt_tiles, wt_tiles, md_t, mh_t, bvb_t, eps_t,
               gpool, hpool, opool, ln, psg, psp, out):
    dt = mybir.dt.bfloat16
    f32 = mybir.dt.float32
    sub = mybir.AluOpType.subtract
    mult = mybir.AluOpType.mult
    add = mybir.AluOpType.add

    g_tiles = [None] * TB
    h_tiles = [None] * TB
    for i in range(TB + 2):
        if i < TB:
            # g'[i] = x-tile @ W^T + b  (bias added during PSUM evacuation).
            # Within a pair, both halves of each dc share one weight load
            # (the _dedup_ldweights pass strips the redundant loads).
            g = gpool.tile([PT, H], dt, tag="g")
            for pair in range(2):
                pg = psg.tile([PT, 2 * HN], f32, tag="pg")
                for dc in range(DC):
                    for half in range(2):
                        hc = pair * 2 + half
                        nc.tensor.matmul(
                            pg[:, half * HN:(half + 1) * HN],
                            xt_tiles[i][:, dc, :],
                            wt_tiles[dc][:, hc * HN:(hc + 1) * HN],
                            start=(dc == 0), stop=(dc == DC - 1))
                nc.vector.tensor_tensor(
                    out=g[:, pair * 2 * HN:(pair + 1) * 2 * HN], in0=pg,
                    in1=bvb_t[:, pair * 2 * HN:(pair + 1) * 2 * HN], op=add)
            g_tiles[i] = g
            # Halo copies sourcing g'[i], issued as early as possible so the
            # SWDGE descriptor-gen latency hides under a full tile period.
            # Halo rows live twice in h_t (bases 0 and HB=64) so the halo
            # matmuls of two H-chunks run concurrently as a row-tiled pair.
            if i + 1 < TB:
                h_next = hpool.tile([H2P, H], dt, tag="halo")
                h_tiles[i + 1] = h_next
                tail = g[PT - MEM:PT, :]
                nc.gpsimd.dma_start(out=h_next[0:MEM, :], in_=tail)
                if PAIR_HALO:
                    nc.gpsimd.dma_start(out=h_next[HB:HB + MEM, :], in_=tail)
            if i == 0:
                h0 = hpool.tile([H2P, H], dt, tag="halo")
                h_tiles[0] = h0
                nc.gpsimd.memset(h0[0:MEM, :], 0.0)
                if PAIR_HALO:
                    nc.gpsimd.memset(h0[HB:HB + MEM, :], 0.0)
            if i >= 1:
                head = g[0:LA, :]
                nc.gpsimd.dma_start(out=h_tiles[i - 1][MEM:HALO, :], in_=head)
                if PAIR_HALO:
                    nc.gpsimd.dma_start(
                        out=h_tiles[i - 1][HB + MEM:H2P, :], in_=head)
            # j == TB-1: rows [MEM:HALO] keep finite stale data from the
            # pool's previous use (always DMA-written first: bufs ring);
            # their mhalo weights are zero so they contribute nothing.
        if i >= 2:
            # band for tile j: pre = Mdiag_j @ g'[j] + Mhalo_j @ halo_j
            j = i - 2
            h_t = h_tiles[j]
            pre_ps = []
            for hc in range(HC):
                pre = psp.tile([PT, HN], f32, tag=f"pre{hc}")
                nc.tensor.matmul(
                    pre, md_t[:, j, :],
                    g_tiles[j][:, hc * HN:(hc + 1) * HN],
                    start=True, stop=False)
                pre_ps.append(pre)
            for hc in range(HC):
                base = (0 if hc % 2 == 0 else HB) if PAIR_HALO else 0
                kw = {"tile_position": (base, 0)} if PAIR_HALO else {}
                nc.tensor.matmul(
                    pre_ps[hc], mh_t[base:base + HALO, j, :],
                    h_t[base:base + HALO, hc * HN:(hc + 1) * HN],
                    start=False, stop=True, **kw)
            # LayerNorm: evacuate PSUM on ScalarE, stats+apply on DVE.
            stats = ln.tile([PT, HC, 6], f32, tag="stats")
            presb_ch = []
            for hc in range(HC):
                pre_sb = opool.tile([PT, HN], f32, tag=f"presb{hc}")
                nc.scalar.copy(out=pre_sb, in_=pre_ps[hc])
                nc.vector.bn_stats(out=stats[:, hc, :], in_=pre_sb)
                presb_ch.append(pre_sb)
            mv = ln.tile([PT, 2], f32, tag="mv")
            nc.vector.bn_aggr(out=mv, in_=stats)
            rstd = ln.tile([PT, 1], f32, tag="rstd")
            nc.scalar.activation(
                out=rstd, in_=mv[:, 1:2],
                func=mybir.ActivationFunctionType.Sqrt,
                bias=eps_t, scale=1.0)
            nc.vector.reciprocal(out=rstd, in_=rstd)
            for hc in range(HC):
                o = opool.tile([PT, HN], f32, tag=f"o{hc}")
                nc.vector.tensor_scalar(
                    out=o, in0=presb_ch[hc],
                    scalar1=mv[:, 0:1], scalar2=rstd,
                    op0=sub, op1=mult)
                eng = nc.sync if ((j + hc) % 2 == 0) else nc.scalar
                eng.dma_start(
                    out=out[j * PT:(j + 1) * PT, hc * HN:(hc + 1) * HN],
                    in_=o)


def _get_runner(reps=1):
    """Compile once; return (run_fn, in_names, out_names).

    run_fn takes a list of global (concatenated-over-cores) jax/np arrays in
    in_names order followed by zero output buffers, returns global outputs.
    Mirrors concourse.bass2jax.run_bass_via_pjrt's multi-core branch, but
    keeps the jitted callable so repeated invocations don't rebuild/retrace.
    """
    key = ("runner", reps)
    if key in _cached:
        return _cached[key]

    import jax
    from jax.experimental.shard_map import shard_map
    from jax.sharding import Mesh, PartitionSpec
    import concourse.mybir as mybir
    from concourse import bass2jax

    if isinstance(reps, tuple):  # ("loop", K): hardware For_i timing variant
        nc = _build_nc(loop_k=reps[1])
    else:
        nc = _build_nc(reps)
    bass2jax.install_neuronx_cc_hook()

    partition_name = nc.partition_id_tensor.name if nc.partition_id_tensor else None
    in_names, out_names, out_avals, zero_outs = [], [], [], []
    for alloc in nc.m.functions[0].allocations:
        if not isinstance(alloc, mybir.MemoryLocationSet):
            continue
        name = alloc.memorylocations[0].name
        if alloc.kind == "ExternalInput":
            if name != partition_name:
                in_names.append(name)
        elif alloc.kind == "ExternalOutput":
            out_names.append(name)
            shape = tuple(alloc.tensor_shape)
            dtype = mybir.dt.np(alloc.dtype)
            out_avals.append(jax.core.ShapedArray(shape, dtype))
            zero_outs.append(np.zeros(shape, dtype))
    n_params = len(in_names)
    all_names = in_names + out_names
    if partition_name is not None:
        all_names.append(partition_name)

    def _body(*args):
        operands = list(args)
        if partition_name is not None:
            operands.append(bass2jax.partition_id_tensor())
        outs = bass2jax._bass_exec_p.bind(
            *operands,
            out_avals=tuple(out_avals),
            in_names=tuple(all_names),
            out_names=tuple(out_names),
            lowering_input_output_aliases=(),
            sim_require_finite=True,
            sim_require_nnan=True,
            nc=nc,
        )
        return tuple(outs)

    devices = jax.devices()[:NCORES]
    assert len(devices) == NCORES, f"need {NCORES} devices, have {len(jax.devices())}"
    mesh = Mesh(np.asarray(devices), ("core",))
    n_outs = len(out_names)
    fn = jax.jit(shard_map(
        _body, mesh=mesh,
        in_specs=(PartitionSpec("core"),) * (n_params + n_outs),
        out_specs=(PartitionSpec("core"),) * n_outs,
        check_rep=False))

    _cached[key] = (fn, in_names, out_names, zero_outs, mesh)
    return _cached[key]


def _prepare_in_arrays(x, W_lin, b_lin, wm, wf):
    """Host prep: per-core inputs concatenated over the core axis (axis 0)."""
    bf16 = ml_dtypes.bfloat16
    M = _band_matrix(wm, wf)
    mdiag, mhalo = _band_blocks(M)
    per_core = {
        "wT": np.ascontiguousarray(W_lin.T).astype(bf16),
        "mdg": mdiag.astype(bf16),
        "mhl": mhalo.astype(bf16),
        "bvb": np.broadcast_to(b_lin.reshape(1, H), (PT, H)).astype(bf16),
    }
    arrays = {}
    # x: per-core transposed, t-tile-major: [B, TB, D, PT]
    xt = np.ascontiguousarray(
        x.reshape(B, TB, PT, D).transpose(0, 1, 3, 2)).astype(bf16)
    arrays["xtT"] = xt.reshape(B * TB, D, PT)
    for name, arr in per_core.items():
        arrays[name] = np.concatenate([arr] * NCORES, axis=0)
    return arrays


def _run(arrays):
    fn, in_names, out_names, zero_outs, _ = _get_runner()
    global_zero = [np.concatenate([z] * NCORES, axis=0) for z in zero_outs]
    args = [arrays[n] for n in in_names] + global_zero
    outs = fn(*args)
    return {n: np.asarray(o) for n, o in zip(out_names, outs)}


def kernel(x, W_lin, b_lin, mem_w, la_w, gamma, beta):
    x = np.asarray(x, np.float32)
    W_lin = np.asarray(W_lin, np.float32)
    b_lin = np.asarray(b_lin, np.float32)
    wm = np.asarray(mem_w, np.float32).sum(axis=-1, dtype=np.float32)
    wf = np.asarray(la_w, np.float32).sum(axis=-1, dtype=np.float32)
    gamma = np.asarray(gamma, np.float32)
    beta = np.asarray(beta, np.float32)

    arrays = _prepare_in_arrays(x, W_lin, b_lin, wm, wf)
    outs = _run(arrays)
    out = outs["out"].reshape(NCORES, L, H)

    # gamma/beta affine (trivial for the spec's ones/zeros fills; exact in general)
    if not np.all(gamma == 1.0):
        out = out * gamma[None, None, :]
    if not np.all(beta == 0.0):
        out = out + beta[None, None, :]
    return np.ascontiguousarray(out.astype(np.float32))


# revision 31
# speedup vs baseline: 1.0311x; 1.0311x over previous
"""DFSMN layer Trainium2 kernel (8-core SPMD, batch-parallel).

Math: per batch b,
  h = x @ W^T + b_lin                      [L, H]
  out_pre[t] = h[t] + mem[t] + fut[t]  ==  (M @ h)[t]
    with M [L, L] banded: identity + past taps (50) + future taps (5),
    taps are scalars per lag: wm = mem_w.sum(-1), wf = la_w.sum(-1).
  out = LayerNorm_H(out_pre) * gamma + beta

On device (per core = one batch):
  g' = x @ W^T + 1 (x) b_lin   (bf16 TensorE fp32 PSUM; bias folded in during
                                PSUM evacuation so M @ g' = M @ h exactly)
  pre[j] = Mdiag_j @ g'[j] + Mhalo_j @ halo_j
      halo_j = rows [128j-50 .. 128j-1] ++ [128(j+1) .. 128(j+1)+4] of g',
      assembled from neighbor tiles by two SBUF->SBUF DMAs. 2 matmuls per
      (tile, H-chunk) instead of the naive 3 band blocks + bias rank-1.
  out = (pre - mean) * rsqrt(var + eps)  (DVE bn_stats/bn_aggr)
"""
import numpy as np
import ml_dtypes

MEM, LA, EPS = 50, 5, 1e-5
B, L, D, H = 8, 2048, 1024, 2048
NCORES = 8
PT = 128              # time tile (partition dim)
TB = L // PT          # 16 time tiles
DC = D // PT          # 8 contract chunks
HN = 512              # matmul moving free dim (one PSUM bank fp32)
HC = H // HN          # 4 H chunks
HALO = MEM + LA       # 55 halo rows per tile
PAIR_HALO = False     # run halo matmuls as concurrent row-tiled pairs
DEDUP_LDW = False     # strip redundant ldweights (measured slower on hw)
HB = 64               # halo copy B partition base (row-tiled matmul pairing)
H2P = HB + HALO if PAIR_HALO else HALO  # halo tile partition count

_cached = {}
last_exec_time_ns = None


def _band_matrix(wm, wf):
    """M [L, L] fp32: out_pre = M @ h."""
    M = np.zeros((L, L), np.float32)
    idx = np.arange(L)
    M[idx, idx] = 1.0
    for t in range(L):
        if t < MEM:
            M[t, :t] += wm[:t]
        else:
            M[t, t - MEM:t] += wm
        hi = min(t + LA, L - 1)
        if hi >= t + 1:
            M[t, t + 1:hi + 1] += wf[:hi - t]
    return M


def _band_blocks(M):
    """Split M into per-tile diag blocks and halo blocks (both as lhsT).

    mdiag[k, j, t] = M[128j + t, 128j + k]          (K=128 rows of tile j)
    mhalo[k, j, t] = M[128j + t, col(j, k)] where
       col = 128j - 50 + k      for k < 50   (past halo; 0 if col < 0)
       col = 128(j+1) + k - 50  for k >= 50  (future halo; 0 if col >= L)
    """
    mdiag = np.zeros((PT, TB, PT), np.float32)
    mhalo = np.zeros((H2P, TB, PT), np.float32)
    for j in range(TB):
        r0 = j * PT
        mdiag[:, j, :] = M[r0:r0 + PT, r0:r0 + PT].T
        for k in range(MEM):
            c = r0 - MEM + k
            if c >= 0:
                mhalo[k, j, :] = M[r0:r0 + PT, c]
        for k in range(LA):
            c = (j + 1) * PT + k
            if c < L:
                mhalo[MEM + k, j, :] = M[r0:r0 + PT, c]
        # duplicate taps at partition base HB: halo matmuls for two H-chunks
        # run as a row-tiled pair (rows 0-63 / 64-127 of the PE array)
        if PAIR_HALO:
            mhalo[HB:HB + HALO, j, :] = mhalo[0:HALO, j, :]
    return mdiag, mhalo


def _dedup_ldweights(nc, mybir):
    """Remove InstLdweights that reload the stationary operand already in
    the PE array (same weights AP, only non-transpose matmuls in between).

    The tile scheduler emits one Ldweights per matmul with no reuse pass
    (walrus's ldw-opt is disabled), so back-to-back matmuls sharing a
    stationary tile pay a redundant ~53ns weight load each. Walrus emits
    non-self-loading matmuls when the explicit Ldweights is absent, so
    deleting the redundant loads is safe: only removes loads with no
    semaphore activity, and moves their scheduling deps onto the following
    matmul (which occupies the same program position).
    """
    removed = 0
    name_map = {}
    for blk in nc.m.functions[0].blocks:
        insts = list(blk.instructions)
        keep = []
        last_key = None
        pending = None  # removed LDW awaiting its following matmul
        for inst in insts:
            if isinstance(inst, mybir.InstLdweights):
                si = inst.sync_info
                clean = si is None or (not si.on_wait and not si.on_update)
                key = str(inst.ins[0]) + str(inst.tile_position)
                if (key == last_key and clean and not inst.is_transpose
                        and pending is None):
                    pending = inst
                    removed += 1
                    continue
                last_key = key
            elif isinstance(inst, mybir.InstMatmult):
                if inst.is_transpose:
                    last_key = None
                if pending is not None:
                    inst.add_sync_dependencies_from(
                        pending.sync_dependency_set_copy())
                    inst.add_nosync_dependencies_from(
                        pending.nosync_dependency_set_copy())
                    name_map[pending.name] = inst.name
                    pending = None
            elif inst.engine == mybir.EngineType.PE:
                # unknown PE-engine op: conservatively assume it can clobber
                # the loaded stationary operand
                last_key = None
                assert pending is None, "unpaired removed ldweights"
            # instructions on other engines can't touch the PE weight array
            keep.append(inst)
        assert pending is None
        if len(keep) != len(insts):
            blk.instructions[:] = keep
    if name_map:
        for blk in nc.m.functions[0].blocks:
            for inst in blk.instructions:
                inst.remap_dependency_names(name_map)
    return removed


def _build_nc(reps=1, loop_k=None):
    from concourse import bacc
    import concourse.mybir as mybir
    import concourse.tile as tile

    dt = mybir.dt.bfloat16
    f32 = mybir.dt.float32

    nc = bacc.Bacc(None, target_bir_lowering=False)
    # x shipped transposed and t-tile-major: [TB, D, PT] so tile i's lhsT
    # slices are one small contiguous region per (i, dc).
    xtT = nc.declare_dram_parameter("xtT", [TB, D, PT], dt, isOutput=False)
    wT = nc.declare_dram_parameter("wT", [D, H], dt, isOutput=False)
    mdg = nc.declare_dram_parameter("mdg", [PT, TB, PT], dt, isOutput=False)
    mhl = nc.declare_dram_parameter("mhl", [H2P, TB, PT], dt, isOutput=False)
    bvb = nc.declare_dram_parameter("bvb", [PT, H], dt, isOutput=False)
    out = nc.declare_dram_parameter("out", [L, H], f32, isOutput=True)

    with tile.TileContext(nc) as tc:
        with tc.tile_pool(name="const", bufs=1) as const, \
             tc.tile_pool(name="gpool", bufs=6) as gpool, \
             tc.tile_pool(name="hpool", bufs=5) as hpool, \
             tc.tile_pool(name="opool", bufs=2) as opool, \
             tc.tile_pool(name="ln", bufs=2) as ln, \
             tc.tile_pool(name="psg", bufs=2, space="PSUM") as psg, \
             tc.tile_pool(name="psp", bufs=1, space="PSUM") as psp:

            # Input DMAs interleaved across the two hwdge queues in
            # first-use order (wt_dc used by tile 0's dc-th matmul, xt_i by
            # tile i) so early matmuls aren't starved by later loads.
            wt_tiles = [const.tile([PT, H], dt, tag=f"wt{dc}", name=f"wt{dc}")
                        for dc in range(DC)]
            xt_tiles = [const.tile([PT, DC, PT], dt, tag=f"xt{i}",
                                   name=f"xt{i}")
                        for i in range(TB)]
            bvb_t = const.tile([PT, H], dt, tag="bvb")
            md_t = const.tile([PT, TB, PT], dt, tag="mdg")
            mh_t = const.tile([H2P, TB, PT], dt, tag="mhl")

            def load_wt(eng, dc):
                eng.dma_start(out=wt_tiles[dc],
                              in_=wT[dc * PT:(dc + 1) * PT, :])

            def load_xt(eng, i):
                eng.dma_start(
                    out=xt_tiles[i],
                    in_=xtT[i].rearrange("(dc p) t -> p dc t", p=PT))

            load_xt(nc.scalar, 0)
            for dc in range(DC):
                load_wt((nc.sync, nc.scalar)[dc % 2], dc)
            load_xt(nc.sync, 1)
            nc.scalar.dma_start(out=bvb_t, in_=bvb[:, :])
            load_xt(nc.sync, 2)
            nc.scalar.dma_start(out=md_t, in_=mdg[:, :, :])
            load_xt(nc.sync, 3)
            nc.scalar.dma_start(out=mh_t, in_=mhl[:, :, :])
            for i in range(4, TB):
                load_xt((nc.sync, nc.scalar)[i % 2], i)
            eps_t = const.tile([PT, 1], f32, tag="eps")
            nc.vector.memset(eps_t, EPS)

            if loop_k is not None:
                with tc.For_i(0, loop_k, 1):
                    _emit_body(nc, mybir, xt_tiles, wt_tiles, md_t, mh_t,
                               bvb_t, eps_t, gpool, hpool, opool, ln, psg,
                               psp, out)
            else:
                for _rep in range(reps):
                    _emit_body(nc, mybir, xt_tiles, wt_tiles, md_t, mh_t,
                               bvb_t, eps_t, gpool, hpool, opool, ln, psg,
                               psp, out)
    if DEDUP_LDW:
        _dedup_ldweights(nc, mybir)
    nc.finalize()
    return nc


def _emit_body(nc, mybir, xt_tiles, wt_tiles, md_t, mh_t, bvb_t, eps_t,
               gpool, hpool, opool, ln, psg, psp, out):
    dt = mybir.dt.bfloat16
    f32 = mybir.dt.float32
    sub = mybir.AluOpType.subtract
    mult = mybir.AluOpType.mult
    add = mybir.AluOpType.add

    g_tiles = [None] * TB
    h_tiles = [None] * TB
    for i in range(TB + 2):
        if i < TB:
            # g'[i] = x-tile @ W^T + b  (bias added during PSUM evacuation).
            # Within a pair, both halves of each dc share one weight load
            # (the _dedup_ldweights pass strips the redundant loads).
            g = gpool.tile([PT, H], dt, tag="g")
            for pair in range(2):
                pg = psg.tile([PT, 2 * HN], f32, tag="pg")
                for dc in range(DC):
                    for half in range(2):
                        hc = pair * 2 + half
                        nc.tensor.matmul(
                            pg[:, half * HN:(half + 1) * HN],
                            xt_tiles[i][:, dc, :],
                            wt_tiles[dc][:, hc * HN:(hc + 1) * HN],
                            start=(dc == 0), stop=(dc == DC - 1))
                nc.vector.tensor_tensor(
                    out=g[:, pair * 2 * HN:(pair + 1) * 2 * HN], in0=pg,
                    in1=bvb_t[:, pair * 2 * HN:(pair + 1) * 2 * HN], op=add)
            g_tiles[i] = g
            # Halo copies sourcing g'[i], issued as early as possible so the
            # SWDGE descriptor-gen latency hides under a full tile period.
            # Halo rows live twice in h_t (bases 0 and HB=64) so the halo
            # matmuls of two H-chunks run concurrently as a row-tiled pair.
            if i + 1 < TB:
                h_next = hpool.tile([H2P, H], dt, tag="halo")
                h_tiles[i + 1] = h_next
                tail = g[PT - MEM:PT, :]
                nc.gpsimd.dma_start(out=h_next[0:MEM, :], in_=tail)
                if PAIR_HALO:
                    nc.gpsimd.dma_start(out=h_next[HB:HB + MEM, :], in_=tail)
            if i == 0:
                h0 = hpool.tile([H2P, H], dt, tag="halo")
                h_tiles[0] = h0
                nc.gpsimd.memset(h0[0:MEM, :], 0.0)
                if PAIR_HALO:
                    nc.gpsimd.memset(h0[HB:HB + MEM, :], 0.0)
            if i >= 1:
                head = g[0:LA, :]
                nc.gpsimd.dma_start(out=h_tiles[i - 1][MEM:HALO, :], in_=head)
                if PAIR_HALO:
                    nc.gpsimd.dma_start(
                        out=h_tiles[i - 1][HB + MEM:H2P, :], in_=head)
            # j == TB-1: rows [MEM:HALO] keep finite stale data from the
            # pool's previous use (always DMA-written first: bufs ring);
            # their mhalo weights are zero so they contribute nothing.
        if i >= 2:
            # band for tile j: pre = Mdiag_j @ g'[j] + Mhalo_j @ halo_j
            j = i - 2
            h_t = h_tiles[j]
            pre_ps = []
            for hc in range(HC):
                pre = psp.tile([PT, HN], f32, tag=f"pre{hc}")
                nc.tensor.matmul(
                    pre, md_t[:, j, :],
                    g_tiles[j][:, hc * HN:(hc + 1) * HN],
                    start=True, stop=False)
                pre_ps.append(pre)
            for hc in range(HC):
                base = (0 if hc % 2 == 0 else HB) if PAIR_HALO else 0
                kw = {"tile_position": (base, 0)} if PAIR_HALO else {}
                nc.tensor.matmul(
                    pre_ps[hc], mh_t[base:base + HALO, j, :],
                    h_t[base:base + HALO, hc * HN:(hc + 1) * HN],
                    start=False, stop=True, **kw)
            # LayerNorm: evacuate PSUM on ScalarE, stats+apply on DVE.
            stats = ln.tile([PT, HC, 6], f32, tag="stats")
            presb_ch = []
            for hc in range(HC):
                pre_sb = opool.tile([PT, HN], f32, tag=f"presb{hc}")
                nc.scalar.copy(out=pre_sb, in_=pre_ps[hc])
                nc.vector.bn_stats(out=stats[:, hc, :], in_=pre_sb)
                presb_ch.append(pre_sb)
            mv = ln.tile([PT, 2], f32, tag="mv")
            nc.vector.bn_aggr(out=mv, in_=stats)
            rstd = ln.tile([PT, 1], f32, tag="rstd")
            nc.scalar.activation(
                out=rstd, in_=mv[:, 1:2],
                func=mybir.ActivationFunctionType.Sqrt,
                bias=eps_t, scale=1.0)
            nc.vector.reciprocal(out=rstd, in_=rstd)
            for hc in range(HC):
                o = opool.tile([PT, HN], f32, tag=f"o{hc}")
                nc.vector.tensor_scalar(
                    out=o, in0=presb_ch[hc],
                    scalar1=mv[:, 0:1], scalar2=rstd,
                    op0=sub, op1=mult)
                eng = nc.sync if ((j + hc) % 2 == 0) else nc.scalar
                eng.dma_start(
                    out=out[j * PT:(j + 1) * PT, hc * HN:(hc + 1) * HN],
                    in_=o)


def _get_runner(reps=1):
    """Compile once; return (run_fn, in_names, out_names).

    run_fn takes a list of global (concatenated-over-cores) jax/np arrays in
    in_names order followed by zero output buffers, returns global outputs.
    Mirrors concourse.bass2jax.run_bass_via_pjrt's multi-core branch, but
    keeps the jitted callable so repeated invocations don't rebuild/retrace.
    """
    key = ("runner", reps)
    if key in _cached:
        return _cached[key]

    import jax
    from jax.experimental.shard_map import shard_map
    from jax.sharding import Mesh, PartitionSpec
    import concourse.mybir as mybir
    from concourse import bass2jax

    if isinstance(reps, tuple):  # ("loop", K): hardware For_i timing variant
        nc = _build_nc(loop_k=reps[1])
    else:
        nc = _build_nc(reps)
    bass2jax.install_neuronx_cc_hook()

    partition_name = nc.partition_id_tensor.name if nc.partition_id_tensor else None
    in_names, out_names, out_avals, zero_outs = [], [], [], []
    for alloc in nc.m.functions[0].allocations:
        if not isinstance(alloc, mybir.MemoryLocationSet):
            continue
        name = alloc.memorylocations[0].name
        if alloc.kind == "ExternalInput":
            if name != partition_name:
                in_names.append(name)
        elif alloc.kind == "ExternalOutput":
            out_names.append(name)
            shape = tuple(alloc.tensor_shape)
            dtype = mybir.dt.np(alloc.dtype)
            out_avals.append(jax.core.ShapedArray(shape, dtype))
            zero_outs.append(np.zeros(shape, dtype))
    n_params = len(in_names)
    all_names = in_names + out_names
    if partition_name is not None:
        all_names.append(partition_name)

    def _body(*args):
        operands = list(args)
        if partition_name is not None:
            operands.append(bass2jax.partition_id_tensor())
        outs = bass2jax._bass_exec_p.bind(
            *operands,
            out_avals=tuple(out_avals),
            in_names=tuple(all_names),
            out_names=tuple(out_names),
            lowering_input_output_aliases=(),
            sim_require_finite=True,
            sim_require_nnan=True,
            nc=nc,
        )
        return tuple(outs)

    devices = jax.devices()[:NCORES]
    assert len(devices) == NCORES, f"need {NCORES} devices, have {len(jax.devices())}"
    mesh = Mesh(np.asarray(devices), ("core",))
    n_outs = len(out_names)
    fn = jax.jit(shard_map(
        _body, mesh=mesh,
        in_specs=(PartitionSpec("core"),) * (n_params + n_outs),
        out_specs=(PartitionSpec("core"),) * n_outs,
        check_rep=False))

    _cached[key] = (fn, in_names, out_names, zero_outs, mesh)
    return _cached[key]


def _prepare_in_arrays(x, W_lin, b_lin, wm, wf):
    """Host prep: per-core inputs concatenated over the core axis (axis 0)."""
    bf16 = ml_dtypes.bfloat16
    M = _band_matrix(wm, wf)
    mdiag, mhalo = _band_blocks(M)
    per_core = {
        "wT": np.ascontiguousarray(W_lin.T).astype(bf16),
        "mdg": mdiag.astype(bf16),
        "mhl": mhalo.astype(bf16),
        "bvb": np.broadcast_to(b_lin.reshape(1, H), (PT, H)).astype(bf16),
    }
    arrays = {}
    # x: per-core transposed, t-tile-major: [B, TB, D, PT]
    xt = np.ascontiguousarray(
        x.reshape(B, TB, PT, D).transpose(0, 1, 3, 2)).astype(bf16)
    arrays["xtT"] = xt.reshape(B * TB, D, PT)
    for name, arr in per_core.items():
        arrays[name] = np.concatenate([arr] * NCORES, axis=0)
    return arrays


def _run(arrays):
    fn, in_names, out_names, zero_outs, _ = _get_runner()
    global_zero = [np.concatenate([z] * NCORES, axis=0) for z in zero_outs]
    args = [arrays[n] for n in in_names] + global_zero
    outs = fn(*args)
    return {n: np.asarray(o) for n, o in zip(out_names, outs)}


def kernel(x, W_lin, b_lin, mem_w, la_w, gamma, beta):
    x = np.asarray(x, np.float32)
    W_lin = np.asarray(W_lin, np.float32)
    b_lin = np.asarray(b_lin, np.float32)
    wm = np.asarray(mem_w, np.float32).sum(axis=-1, dtype=np.float32)
    wf = np.asarray(la_w, np.float32).sum(axis=-1, dtype=np.float32)
    gamma = np.asarray(gamma, np.float32)
    beta = np.asarray(beta, np.float32)

    arrays = _prepare_in_arrays(x, W_lin, b_lin, wm, wf)
    outs = _run(arrays)
    out = outs["out"].reshape(NCORES, L, H)

    # gamma/beta affine (trivial for the spec's ones/zeros fills; exact in general)
    if not np.all(gamma == 1.0):
        out = out * gamma[None, None, :]
    if not np.all(beta == 0.0):
        out = out + beta[None, None, :]
    return np.ascontiguousarray(out.astype(np.float32))
